# revision 28
# baseline (speedup 1.0000x reference)
"""Trainium2 Bass kernel for GQA sliding-window attention with logit soft-cap.

Problem: B=2, T=2048, D=3584, N=16 q-heads, K=8 kv-heads, H=256,
sliding window 1024, causal, soft-cap 50, query scale 0.0625, RoPE.

Sharding: 8 cores = 2 (batch) x 4 (head groups). Each core handles one
batch and 4 q-heads / 2 kv-heads (tensor-parallel on the head axis of
q_w/kv_w/out_w). Host sums the 4 partial out-projections per batch.

On-device dataflow (all matmuls float32r = tf32-like at full PE rate):
  P1: qT/kT/vT = (x @ W)^T per head, RoPE fused on q/k ([H,T] layout).
  P2: transposed flash-attention-free softmax: logits^T tiles [k,q],
      p = exp(50*tanh(L) - 50) (soft-cap bound makes a running max
      unnecessary), PV and column-sums via ones-matmul accumulate in
      PSUM, normalize with partition-broadcast reciprocal -> encT.
  P3: out = enc @ out_w accumulated over the core's 8 (head,hc) chunks.
"""

import os
import sys

sys.path.insert(0, "/opt/trn_rl_repo")

import numpy as np

B, T, D = 2, 2048, 3584
NQ, NKV, H = 16, 8, 256
P = 128
DC = D // P                 # 28 contraction chunks
HEADS_PER_CORE = 4          # q heads per core
KV_PER_CORE = 2
SOFT_CAP = 50.0
SCALE = 0.0625
WINDOW = 1024
BASE_FREQ = 10000.0
QTILE = 512                 # q-tile width in attention / T-chunk in P1
NQT = T // QTILE            # 4
NKT = T // P                # 16 k-tiles
MASK_NEG = -4.0             # added to tanh output; exp(50*(t-4)-50) == 0

_NC_CACHE = {}
LAST_RESULTS = None         # BassKernelResults of the last run (for test.py)


def _kt_list(qt):
    """Valid k-tiles for q-block qt, with mask index (None = fully allowed).

    mask idx 0-3: causal-diagonal patterns, rel = K0-Q0 in {0,128,256,384}
    mask idx 4-7: window-edge patterns,  w = Q0-K0-1024 in {0,-128,-256,-384}
    """
    Q0 = qt * QTILE
    out = []
    for kt in range(NKT):
        K0 = kt * P
        if K0 > Q0 + QTILE - 1:          # fully above the diagonal
            continue
        if K0 + P - 1 <= Q0 - WINDOW:    # fully outside the window
            continue
        rel = K0 - Q0
        if rel >= 0:
            out.append((kt, rel // P))
        else:
            w = Q0 - K0 - WINDOW
            if -QTILE < w <= 0:
                out.append((kt, 4 + (-w) // P))
            else:
                out.append((kt, None))
    return out


def _make_masks():
    m = np.zeros((8, P, QTILE), np.float32)
    i = np.arange(P)[:, None]
    j = np.arange(QTILE)[None, :]
    for r in range(4):           # diag: allowed iff i <= j - rel
        m[r] = np.where(i <= j - r * P, 1.0, 0.0)
    for wi in range(4):          # window: allowed iff i > j + w
        w = -wi * P
        m[4 + wi] = np.where(i > j + w, 1.0, 0.0)
    return m


def _build_nc():
    import concourse.bacc as bacc
    import concourse.mybir as mybir
    import concourse.tile as tile
    from concourse.masks import make_identity

    f32 = mybir.dt.float32
    f32r = mybir.dt.float32r
    AF = mybir.ActivationFunctionType

    nc = bacc.Bacc()
    # weights/rope/masks are pre-arranged on the host so every DMA reads
    # large contiguous spans per partition (descriptor-count matters)
    xT = nc.dram_tensor("xT", (D, T), f32r, kind="ExternalInput")
    qw = nc.dram_tensor("qw", (HEADS_PER_CORE, P, DC, H), f32r, kind="ExternalInput")
    kw = nc.dram_tensor("kw", (KV_PER_CORE, P, DC, H), f32r, kind="ExternalInput")
    vw = nc.dram_tensor("vw", (KV_PER_CORE, P, DC, H), f32r, kind="ExternalInput")
    ow = nc.dram_tensor("ow", (HEADS_PER_CORE, H, D), f32r, kind="ExternalInput")
    rope = nc.dram_tensor("rope", (P, 4, T), f32, kind="ExternalInput")
    msk = nc.dram_tensor("msk", (P, 8, QTILE), f32, kind="ExternalInput")
    out = nc.dram_tensor("out", (T, D), f32, kind="ExternalOutput")

    xTr = xT.rearrange("(c p) t -> p c t", p=P)

    with tile.TileContext(nc) as tc:
        with tc.tile_pool(name="dram", bufs=1, space="DRAM") as dpool:
            qT = dpool.tile([HEADS_PER_CORE, 2, P, T], f32r)
            kT = dpool.tile([KV_PER_CORE, 2, P, T], f32r)
            vT = dpool.tile([KV_PER_CORE, 2, P, T], f32r)
            eT = dpool.tile([HEADS_PER_CORE, 2, P, T], f32r)

            # ---------------- P1: projections + RoPE ----------------
            with (
                tc.tile_pool(name="p1w", bufs=1) as wpool,
                tc.tile_pool(name="p1x", bufs=8) as xpool,
                tc.tile_pool(name="p1r", bufs=1) as rpool,
                tc.tile_pool(name="p1o", bufs=3) as opool,
                tc.tile_pool(name="p1ps", bufs=1, space="PSUM") as pspool,
            ):
                rope_sb = rpool.tile([P, 4, T], f32)
                nc.sync.dma_start(rope_sb[:], rope[:])
                for half in range(2):  # 0: k0,k1,v0,v1   1: q0..q3
                    wts = []
                    for j in range(4):
                        wt = wpool.tile([P, DC, H], f32r, tag=f"w{j}")
                        src = qw[j] if half == 1 else (kw[j] if j < 2 else vw[j - 2])
                        nc.sync.dma_start(wt[:], src[:])
                        wts.append(wt)
                    for n in range(NQT):
                        ns = slice(n * QTILE, (n + 1) * QTILE)
                        psums = [
                            [
                                pspool.tile(
                                    [P, QTILE], f32, tag=f"ps{j}{hc}",
                                    name=f"ps{j}{hc}",
                                )
                                for hc in range(2)
                            ]
                            for j in range(4)
                        ]
                        for d in range(DC):
                            xt = xpool.tile([P, QTILE], f32r, tag="xt")
                            nc.sync.dma_start(xt[:], xTr[:, d, ns])
                            for j in range(4):
                                for hc in range(2):
                                    nc.tensor.matmul(
                                        psums[j][hc][:],
                                        wts[j][:, d, hc * P : (hc + 1) * P],
                                        xt[:],
                                        start=(d == 0),
                                        stop=(d == DC - 1),
                                    )
                        for j in range(4):
                            if half == 0 and j >= 2:  # v: copy out on idle ACT
                                for hc in range(2):
                                    o = opool.tile([P, QTILE], f32r, tag="vo")
                                    nc.scalar.copy(o[:], psums[j][hc][:])
                                    nc.sync.dma_start(vT[j - 2, hc, :, ns], o[:])
                            else:  # q/k: RoPE (q uses pre-scaled tables)
                                ci, si = (2, 3) if half == 1 else (0, 1)
                                cos_t = rope_sb[:, ci, ns]
                                sin_t = rope_sb[:, si, ns]
                                # drain each PSUM bank with both its reads
                                # back-to-back so banks free ASAP for the
                                # next n-chunk's accumulation
                                c0 = opool.tile([P, QTILE], f32, tag="c0")
                                s0 = opool.tile([P, QTILE], f32, tag="s0")
                                c1 = opool.tile([P, QTILE], f32, tag="c1")
                                s1 = opool.tile([P, QTILE], f32, tag="s1")
                                o0 = opool.tile([P, QTILE], f32r, tag="o0")
                                o1 = opool.tile([P, QTILE], f32r, tag="o1")
                                nc.vector.tensor_mul(c0[:], psums[j][0][:], cos_t)
                                nc.vector.tensor_mul(s0[:], psums[j][0][:], sin_t)
                                nc.vector.tensor_mul(c1[:], psums[j][1][:], cos_t)
                                nc.vector.tensor_mul(s1[:], psums[j][1][:], sin_t)
                                nc.vector.tensor_sub(o0[:], c0[:], s1[:])
                                nc.vector.tensor_add(o1[:], c1[:], s0[:])
                                dstT = qT[j] if half == 1 else kT[j]
                                nc.sync.dma_start(dstT[0, :, ns], o0[:])
                                nc.sync.dma_start(dstT[1, :, ns], o1[:])

            # ---------------- P2: attention ----------------
            with (
                tc.tile_pool(name="p2c", bufs=1) as cpool,
                tc.tile_pool(name="p2kv", bufs=2) as kvpool,
                tc.tile_pool(name="p2q", bufs=2) as qpool,
                tc.tile_pool(name="p2s", bufs=3) as spool,
                tc.tile_pool(name="p2e", bufs=2) as epool,
                tc.tile_pool(name="p2ps", bufs=1, space="PSUM") as pspool,
                tc.tile_pool(name="p2psL", bufs=1, space="PSUM") as psL,
            ):
                masks_sb = cpool.tile([P, 8, QTILE], f32)
                nc.sync.dma_start(masks_sb[:], msk[:])
                ones_f = cpool.tile([P, 1], f32)
                nc.vector.memset(ones_f[:], 1.0)
                ones_r = cpool.tile([P, 1], f32r)
                nc.vector.tensor_copy(ones_r[:], ones_f[:])
                bias_m50 = cpool.tile([P, 1], f32)
                nc.vector.memset(bias_m50[:], -SOFT_CAP)
                idf = cpool.tile([P, P], f32)
                make_identity(nc, idf[:])
                idr = cpool.tile([P, P], f32r)
                nc.vector.tensor_copy(idr[:], idf[:])

                for kvh in range(KV_PER_CORE):
                    kT_sb = kvpool.tile([P, 2, T], f32r, tag="kT")
                    nc.sync.dma_start(kT_sb[:], kT[kvh].rearrange("c p t -> p c t"))
                    vT_sb = kvpool.tile([P, 2, T], f32r, tag="vT")
                    nc.sync.dma_start(vT_sb[:], vT[kvh].rearrange("c p t -> p c t"))
                    v_all = kvpool.tile([P, NKT, H], f32r, tag="va")
                    for kt in range(NKT):
                        for hc in range(2):
                            pst = psL.tile([P, P], f32r, tag="L", name="pst")
                            nc.tensor.transpose(
                                pst[:], vT_sb[:, hc, kt * P : (kt + 1) * P], idr[:]
                            )
                            nc.vector.tensor_copy(
                                v_all[:, kt, hc * P : (hc + 1) * P], pst[:]
                            )
                    for qh in (2 * kvh, 2 * kvh + 1):
                        qT_sb = qpool.tile([P, 2, T], f32r, tag="qT")
                        nc.sync.dma_start(
                            qT_sb[:], qT[qh].rearrange("c p t -> p c t")
                        )
                        for qt in range(NQT):
                            qs = slice(qt * QTILE, (qt + 1) * QTILE)
                            kts = _kt_list(qt)
                            pairs = [
                                (kts[2 * i], kts[2 * i + 1])
                                for i in range(len(kts) // 2)
                            ]
                            db = qt % 2  # double-buffer accumulators
                            enc_ps = [
                                pspool.tile(
                                    [P, QTILE], f32, tag=f"enc{hc}{db}",
                                    name=f"enc{hc}",
                                )
                                for hc in range(2)
                            ]
                            s_ps = pspool.tile(
                                [1, QTILE], f32, tag=f"sums{db}", name="s_ps"
                            )
                            for i, pair in enumerate(pairs):
                                st, sp = (i == 0), (i == len(pairs) - 1)
                                # two k-tiles per L so tanh/exp amortize the
                                # ACT per-instruction overhead
                                L = psL.tile([P, 2 * QTILE], f32, tag="L")
                                for u, (kt, _) in enumerate(pair):
                                    us = slice(u * QTILE, (u + 1) * QTILE)
                                    nc.tensor.matmul(
                                        L[:, us],
                                        kT_sb[:, 0, kt * P : (kt + 1) * P],
                                        qT_sb[:, 0, qs],
                                        start=True,
                                        stop=False,
                                    )
                                    nc.tensor.matmul(
                                        L[:, us],
                                        kT_sb[:, 1, kt * P : (kt + 1) * P],
                                        qT_sb[:, 1, qs],
                                        start=False,
                                        stop=True,
                                    )
                                tt = spool.tile([P, 2 * QTILE], f32, tag="t")
                                nc.scalar.activation(tt[:], L[:], AF.Tanh)
                                pp = spool.tile([P, 2 * QTILE], f32r, tag="p")
                                nc.scalar.activation(
                                    pp[:],
                                    tt[:],
                                    AF.Exp,
                                    bias=bias_m50[:],
                                    scale=SOFT_CAP,
                                )
                                for u, (kt, mi) in enumerate(pair):
                                    us = slice(u * QTILE, (u + 1) * QTILE)
                                    pu = pp[:, us]
                                    if mi is not None:
                                        pm = spool.tile(
                                            [P, QTILE], f32r, tag="pm",
                                            name="pm",
                                        )
                                        nc.vector.tensor_mul(
                                            pm[:], pu, masks_sb[:, mi]
                                        )
                                        pu = pm[:]
                                    nc.tensor.matmul(
                                        enc_ps[0][:],
                                        v_all[:, kt, 0:P],
                                        pu,
                                        start=(st and u == 0),
                                        stop=(sp and u == 1),
                                    )
                                    nc.tensor.matmul(
                                        enc_ps[1][:],
                                        v_all[:, kt, P:H],
                                        pu,
                                        start=(st and u == 0),
                                        stop=(sp and u == 1),
                                    )
                                    nc.tensor.matmul(
                                        s_ps[:],
                                        ones_r[:],
                                        pu,
                                        start=(st and u == 0),
                                        stop=(sp and u == 1),
                                    )
                            rec = spool.tile([1, QTILE], f32, tag="rec")
                            nc.vector.reciprocal(rec[:], s_ps[:])
                            rb = spool.tile([P, QTILE], f32, tag="rb")
                            nc.gpsimd.partition_broadcast(rb[:], rec[:])
                            for hc in range(2):
                                eo = epool.tile([P, QTILE], f32r, tag="eo")
                                nc.vector.tensor_mul(eo[:], enc_ps[hc][:], rb[:])
                                nc.sync.dma_start(eT[qh, hc, :, qs], eo[:])

            # ---------------- P3: output projection ----------------
            with (
                tc.tile_pool(name="p3w", bufs=1) as owpool,
                tc.tile_pool(name="p3e", bufs=2) as e3pool,
                tc.tile_pool(name="p3o", bufs=2) as o3pool,
                tc.tile_pool(name="p3ps", bufs=3, space="PSUM") as pspool,
            ):
                ow_sb = []
                for head in range(HEADS_PER_CORE):
                    for hc in range(2):
                        wt = owpool.tile(
                            [P, D], f32r, tag=f"ow{head}{hc}",
                            name=f"ow{head}{hc}",
                        )
                        nc.sync.dma_start(
                            wt[:], ow[head, hc * P : (hc + 1) * P, :]
                        )
                        ow_sb.append(wt)
                SPAN = 512  # t-columns of eT fetched per load round
                for tci in range(T // P):
                    ts_ = slice(tci * P, (tci + 1) * P)
                    if tci % (SPAN // P) == 0:
                        sp = slice(tci * P, tci * P + SPAN)
                        ets = []
                        for head in range(HEADS_PER_CORE):
                            for hc in range(2):
                                et = e3pool.tile(
                                    [P, SPAN], f32r, tag=f"et{head}{hc}",
                                    name=f"et{head}{hc}",
                                )
                                nc.sync.dma_start(et[:], eT[head, hc, :, sp])
                                ets.append(et)
                    off = (tci % (SPAN // P)) * P
                    lhs = [e[:, off : off + P] for e in ets]
                    out_sb = o3pool.tile([P, D], f32, tag="osb")
                    for nn in range(D // QTILE):
                        nns = slice(nn * QTILE, (nn + 1) * QTILE)
                        po = pspool.tile([P, QTILE], f32, tag="po")
                        for j in range(2 * HEADS_PER_CORE):
                            nc.tensor.matmul(
                                po[:],
                                lhs[j][:],
                                ow_sb[j][:, nns],
                                start=(j == 0),
                                stop=(j == 2 * HEADS_PER_CORE - 1),
                            )
                        if nn % 2 == 0:
                            nc.vector.tensor_copy(out_sb[:, nns], po[:])
                        else:
                            nc.scalar.copy(out_sb[:, nns], po[:])
                    nc.sync.dma_start(out[ts_, :], out_sb[:])

    nc.finalize()
    return nc


def _install_neff_cache():
    """Cache walrus-compiled NEFFs by BIR hash (compiles are minutes-long)."""
    import hashlib
    import shutil

    import concourse.bass2jax as b2j

    if getattr(b2j, "_ant_neff_cache_installed", False):
        return
    orig = b2j.compile_bir_kernel

    def cached(bir_json, tmpdir, neff_name="file.neff"):
        cdir = os.environ.get("NEFF_CACHE_DIR", "/tmp/neff_cache")
        os.makedirs(cdir, exist_ok=True)
        h = hashlib.sha256(bir_json).hexdigest()[:32]
        cpath = os.path.join(cdir, f"{h}.neff")
        if os.path.exists(cpath):
            dst = os.path.join(tmpdir, "sg00")
            os.makedirs(dst, exist_ok=True)
            dstf = os.path.join(dst, neff_name)
            shutil.copyfile(cpath, dstf)
            return dstf
        r = orig(bir_json, tmpdir, neff_name=neff_name)
        try:
            shutil.copyfile(r, cpath)
        except OSError:
            pass
        return r

    b2j.compile_bir_kernel = cached
    b2j._ant_neff_cache_installed = True


def kernel(x, segment_pos, attn_mask, q_w, kv_w, out_w):
    global LAST_RESULTS
    from concourse.bass_utils import run_bass_kernel_spmd

    _install_neff_cache()

    x = np.asarray(x, np.float32)
    segment_pos = np.asarray(segment_pos, np.int32)
    q_w = np.asarray(q_w, np.float32)
    kv_w = np.asarray(kv_w, np.float32)
    out_w = np.asarray(out_w, np.float32)

    # RoPE tables per batch: [cos, sin, cos*s, sin*s] with s = SCALE/SOFT_CAP
    # host layout [P, 4, T] (partition-major contiguous for one big DMA)
    ropes = []
    for b in range(B):
        pos = segment_pos[b].astype(np.float32)
        fraction = 2.0 * np.arange(P, dtype=np.float32) / H
        timescale = BASE_FREQ**fraction
        ang = pos[None, :] / timescale[:, None]          # [128, T]
        s = SCALE / SOFT_CAP
        r = np.stack(
            [np.cos(ang), np.sin(ang), np.cos(ang) * s, np.sin(ang) * s]
        ).astype(np.float32)
        ropes.append(np.ascontiguousarray(r.transpose(1, 0, 2)))
    masks = np.ascontiguousarray(_make_masks().transpose(1, 0, 2))

    def _wlayout(w):
        # [nh, D, H] -> [nh, P, DC, H]: per-partition contiguous weight spans
        return np.ascontiguousarray(
            w.reshape(-1, DC, P, H).transpose(0, 2, 1, 3)
        )

    key = "main"
    if key not in _NC_CACHE:
        _NC_CACHE[key] = _build_nc()
    nc = _NC_CACHE[key]

    in_maps = []
    for core in range(8):
        b, g = core // 4, core % 4
        in_maps.append(
            {
                "xT": np.ascontiguousarray(x[b].T),
                "qw": _wlayout(q_w[4 * g : 4 * g + 4]),
                "kw": _wlayout(kv_w[0, 2 * g : 2 * g + 2]),
                "vw": _wlayout(kv_w[1, 2 * g : 2 * g + 2]),
                "ow": np.ascontiguousarray(out_w[4 * g : 4 * g + 4]),
                "rope": ropes[b],
                "msk": masks,
            }
        )

    res = run_bass_kernel_spmd(nc, in_maps, core_ids=list(range(8)))
    LAST_RESULTS = res

    out = np.zeros((B, T, D), np.float32)
    for core in range(8):
        out[core // 4] += res.results[core]["out"]
    return out
